# revision 34
# baseline (speedup 1.0000x reference)
"""Sliding-window causal GQA self-attention (RoPE + QK-RMSNorm) for TRN2.

Sharding (8 cores): core c = (b, g) with b = c // 4, g = c % 4.
Each core handles batch b, q-heads [4g, 4g+4), kv-head g, and the
column-slice [512g, 512g+512) of the c_proj contraction (row-sharded Wo).
Host sums the 4 partial outputs per batch (the "all-reduce").

v2: all matmul operands in bf16 (rel tol 2e-2 allows it), host-packed
DRAM layouts so every DMA line is >= 512B contiguous, interleaved x/w
loads so PE starts ~4us in, Wo preloaded during P1 on the Pool queue,
softmax denominator via ones[128,128] stationary matmul (same cycles as
ones[128,1] but the result lands broadcast across partitions - no DRAM
round-trip), piece-level scores->exp pipelining, i-major c_proj with one
8KB-line store per token tile.

Per-core device program (T=2048, 16 token tiles of 128, head_dim 128):
  P1: QKV projection (x @ W.T in [tok, feat] layout), RoPE + RMS-norm on
      q/k (batched over the 5 heads), PE-transpose q/k to [d, tok]; v
      stays [tok, d].
  P2: per head: scores^T s[k, q] = K_d^T.T @ Q_d in ~512-wide pieces,
      exp on ACT (scale folded) -> et bf16, boundary masks on DVE,
      PV + broadcast-denominator accumulation, reciprocal + multiply.
  P3: c_proj partial: out[tok, :] = sum_h yT_h.T @ Wo_h, one store/tile.
"""

import math
import os
import sys

sys.path.insert(0, "/opt/trn_rl_repo")

import numpy as np

import concourse.bass as bass
import concourse.mybir as mybir
import concourse.tile as tile
from concourse import bacc
from concourse.bass_utils import run_bass_kernel_spmd

F32 = mybir.dt.float32
BF = mybir.dt.bfloat16
AF = mybir.ActivationFunctionType
ALU = mybir.AluOpType

B, T, C = 2, 2048, 2048
N_HEAD, N_KV_HEAD, D = 16, 4, 128
NT = T // 128          # 16 token tiles
KT = C // 128          # 16 contraction tiles
NH = N_HEAD // 4       # 4 q heads per core
FEAT = NH * D + 2 * D  # 768 projected features per core (q 512 | k 128 | v 128)
EPS = float(np.finfo(np.float32).eps)


def _window_masks(W: int):
    """Per block-offset o = qi - kj: multiplicative mask [k, q] or None if all-valid."""
    omax = max(0, -(-W // 128))  # ceil(W/128)
    k = np.arange(128)[:, None]
    q = np.arange(128)[None, :]
    masks = {}
    for o in range(omax + 1):
        d = q + 128 * o - k
        m = ((d >= 0) & (d <= W)).astype(np.float32)
        if not np.all(m == 1.0):
            masks[o] = m
    return omax, masks


def build_nc(W: int):
    omax, masks = _window_masks(W)
    mask_off = sorted(masks.keys())
    mask_idx = {o: i for i, o in enumerate(mask_off)}
    nm = max(1, len(mask_off))
    max_span = (omax + 1) * 128

    nc = bacc.Bacc(None, target_bir_lowering=False)
    xd = nc.dram_tensor("xd", [NT, 128, KT * 128], BF, kind="ExternalInput")
    wd = nc.dram_tensor("wd", [KT, 128, FEAT], BF, kind="ExternalInput")
    wod = nc.dram_tensor("wod", [128, NH, C], BF, kind="ExternalInput")
    cosd = nc.dram_tensor("cosd", [128, NT, 64], BF, kind="ExternalInput")
    sind = nc.dram_tensor("sind", [128, NT, 64], BF, kind="ExternalInput")
    maskd = nc.dram_tensor("maskd", [128, nm, 128], BF, kind="ExternalInput")
    onesd = nc.dram_tensor("onesd", [128, 128], BF, kind="ExternalInput")
    identd = nc.dram_tensor("identd", [128, 128], BF, kind="ExternalInput")
    outp = nc.dram_tensor("outp", [T, C], F32, kind="ExternalOutput")

    scale = 1.0 / math.sqrt(D)

    with tile.TileContext(nc) as tc:
        with tc.tile_pool(name="persist", bufs=1) as per:
            cos_sb = per.tile([128, NT, 64], BF, tag="cos")
            sin_sb = per.tile([128, NT, 64], BF, tag="sin")
            mask_sb = per.tile([128, nm, 128], BF, tag="mask")
            ones_sb = per.tile([128, 128], BF, tag="ones")
            ident_sb = per.tile([128, 128], BF, tag="ident")
            wo_sb = per.tile([128, NH, C], BF, tag="wo")
            # q/k/v split into token halves so P2 head-0 scores can start
            # while P1's last tiles are still draining (tile-granular deps)
            qdT_a = per.tile([128, NH, T // 2], BF, tag="qdTa")  # [d, h, tok]
            qdT_b = per.tile([128, NH, T // 2], BF, tag="qdTb")
            kdT_a = per.tile([128, T // 2], BF, tag="kdTa")      # [d, tok]
            kdT_b = per.tile([128, T // 2], BF, tag="kdTb")
            vsb_a = per.tile([128, NT // 2, 128], BF, tag="va")  # [tok%128, kj, d]
            vsb_b = per.tile([128, NT // 2, 128], BF, tag="vb")
            # per-k-token softmax scale: rs_k/sqrt(D), folded into the exp
            rsk_sb = per.tile([128, NT], F32, tag="rsk")
            # per-512-token-group yT tiles: P3's early tiles must not wait
            # on the last head's final divide (tile-granular deps)
            yTg = [per.tile([128, NH, 512], BF, name=f"yTg{g}", tag=f"yTg{g}")
                   for g in range(T // 512)]


            def q_pieces(qlo, qhi, h):
                t0, t1 = qlo * 128, (qhi + 1) * 128
                out = []
                for base, tl in ((0, qdT_a), (T // 2, qdT_b)):
                    lo, hi = max(t0, base), min(t1, base + T // 2)
                    pos = lo
                    while pos < hi:
                        n = hi - pos
                        n_p = (n + 511) // 512
                        wdt = n // n_p
                        out.append(
                            (tl[:, h, pos - base:pos - base + wdt],
                             pos - t0, wdt))
                        pos += wdt
                return out

            def emit_scores(h, kj):
                qlo = kj
                qhi = min(kj + omax, NT - 1)
                kd = kdT_a[:, kj * 128:(kj + 1) * 128] if kj < 8 \
                    else kdT_b[:, (kj - 8) * 128:(kj - 7) * 128]
                et = p2e.tile([128, max_span], BF, tag="expT")
                for src_ap, eoff, wdt in q_pieces(qlo, qhi, h):
                    ps = p2ps.tile([128, 512], F32, tag="sT")
                    nc.tensor.matmul(
                        ps[:, 0:wdt], kd, src_ap,
                        start=True, stop=True,
                    )
                    nc.scalar.activation(
                        et[:, eoff:eoff + wdt], ps[:, 0:wdt],
                        AF.Exp, scale=rsk_sb[:, kj:kj + 1])
                for o in mask_off:
                    if qlo + o <= qhi:
                        sl = et[:, o * 128:(o + 1) * 128]
                        nc.vector.tensor_mul(
                            sl, sl, mask_sb[:, mask_idx[o], :])
                return et

            ets0 = {}

            # ---------------- Phase 1: QKV + RoPE + RMS + transpose ----------
            _p2e_cm = tc.tile_pool(name="p2e", bufs=NT)
            _p2s_cm = tc.tile_pool(name="p2s", bufs=2)
            _p2ps_cm = tc.tile_pool(name="p2ps", bufs=3, space="PSUM")
            p2e = _p2e_cm.__enter__()
            p2s = _p2s_cm.__enter__()
            p2ps = _p2ps_cm.__enter__()
            with tc.tile_pool(name="p1w", bufs=1) as p1w, \
                 tc.tile_pool(name="p1x", bufs=3) as p1x, \
                 tc.tile_pool(name="p1s", bufs=2) as p1s, \
                 tc.tile_pool(name="p1ps", bufs=2, space="PSUM") as p1ps, \
                 tc.tile_pool(name="p1pt", bufs=1, space="PSUM") as p1pt:

                wqkv = p1w.tile([128, KT, FEAT], BF, tag="wqkv")
                xks = {}

                def load_x(i, split=False):
                    xk = p1x.tile([128, KT * 128], BF, tag="xk")
                    if split:
                        half = KT * 64
                        nc.sync.dma_start(out=xk[:, 0:half], in_=xd[i, :, 0:half])
                        nc.sync.dma_start(out=xk[:, half:], in_=xd[i, :, half:])
                    else:
                        nc.sync.dma_start(out=xk, in_=xd[i, :, :])
                    xks[i] = xk

                # first w chunk + first x tile lead; then stream the rest of
                # w interleaved with the next few x tiles (tile 0's k-loop
                # consumes w chunks in arrival order)
                for k in range(0, 4):
                    nc.gpsimd.dma_start(out=wqkv[:, k, :], in_=wd[k, :, :])
                load_x(0, split=True)
                for k in range(4, KT):
                    nc.scalar.dma_start(out=wqkv[:, k, :], in_=wd[k, :, :])
                load_x(1)
                load_x(2)
                # persistent state behind the critical w0-3 chunks on the
                # Pool queue: ident first (tile-0 transposes), then rope
                # tables, then P2's masks/ones
                nc.gpsimd.dma_start(out=ident_sb, in_=identd[:, :])
                nc.gpsimd.dma_start(out=cos_sb, in_=cosd[:, :, :])
                nc.gpsimd.dma_start(out=sin_sb, in_=sind[:, :, :])
                nc.gpsimd.dma_start(out=mask_sb, in_=maskd[:, :, :])
                nc.gpsimd.dma_start(out=ones_sb, in_=onesd[:, :])

                # lag-1 software pipeline: tile i's vector chain is emitted
                # after tile i+1's matmuls so PE never waits on DVE/ACT.
                pending = {}

                def p1_tail(j):
                    ps_q, ps_kv = pending.pop(j)
                    # gather q (4 heads) + k into one [128, 5, 128] tile so
                    # the rope/rms chain runs batched (fewer, wider DVE ops)
                    qk5 = p1s.tile([128, 5, 128], BF, tag="qk5")
                    nc.scalar.activation(
                        qk5[:, 0:4, :].rearrange("p h d -> p (h d)"),
                        ps_q, AF.Copy)
                    nc.scalar.activation(qk5[:, 4, :], ps_kv[:, 0:128], AF.Copy)
                    vdst = vsb_a[:, j, :] if j < 8 else vsb_b[:, j - 8, :]
                    nc.scalar.activation(vdst, ps_kv[:, 128:256], AF.Copy)

                    cos5 = cos_sb[:, j, :].unsqueeze(1).broadcast_to([128, 5, 64])
                    sin5 = sin_sb[:, j, :].unsqueeze(1).broadcast_to([128, 5, 64])
                    x1 = qk5[:, :, 0:64]
                    x2 = qk5[:, :, 64:128]
                    rot = p1s.tile([128, 5, 128], BF, tag="rot")
                    ta = p1s.tile([128, 5, 64], BF, tag="ta")
                    tb = p1s.tile([128, 5, 64], BF, tag="tb")
                    tc_ = p1s.tile([128, 5, 64], BF, tag="tc")
                    td = p1s.tile([128, 5, 64], BF, tag="td")
                    nc.vector.tensor_mul(ta, x1, cos5)
                    nc.vector.tensor_mul(tb, x2, sin5)
                    nc.vector.tensor_add(rot[:, :, 0:64], ta, tb)
                    nc.vector.tensor_mul(tc_, x2, cos5)
                    nc.vector.tensor_mul(td, x1, sin5)
                    nc.vector.tensor_sub(rot[:, :, 64:128], tc_, td)

                    # RMS norm scale: rs = 1/sqrt(mean(rot^2) + eps)
                    sq = p1s.tile([128, 5, 128], BF, tag="sq")
                    nc.vector.tensor_mul(sq, rot, rot)
                    ss = p1s.tile([128, 8], F32, tag="ss")
                    nc.vector.tensor_reduce(
                        out=ss[:, 0:5], in_=sq,
                        axis=mybir.AxisListType.X, op=ALU.add,
                    )
                    tt = p1s.tile([128, 8], F32, tag="tt")
                    nc.vector.tensor_scalar(
                        out=tt[:, 0:5], in0=ss[:, 0:5],
                        scalar1=1.0 / D, scalar2=EPS,
                        op0=ALU.mult, op1=ALU.add,
                    )
                    rr = p1s.tile([128, 8], F32, tag="rr")
                    nc.vector.reciprocal(rr[:, 0:5], tt[:, 0:5])
                    rs = p1s.tile([128, 8], F32, tag="rs")
                    nc.scalar.activation(rs[:, 0:4], rr[:, 0:4], AF.Sqrt)
                    # k's rms scale folds into the exp (per-k-partition
                    # scale vector), with the 1/sqrt(D) baked in
                    nc.scalar.activation(
                        rsk_sb[:, j:j + 1], rr[:, 4:5], AF.Sqrt, scale=1.0 / D)

                    qn = p1s.tile([128, 4, 128], BF, tag="qn")
                    for h in range(4):
                        nc.vector.tensor_scalar_mul(
                            qn[:, h, :], rot[:, h, :], rs[:, h:h + 1])
                    qd_j = qdT_a if j < 8 else qdT_b
                    kd_j = kdT_a if j < 8 else kdT_b
                    jj = (j % 8) * 128
                    for h in range(5):
                        pt = p1pt.tile([128, 128], BF, tag="pt")
                        src_t = qn[:, h, :] if h < NH else rot[:, 4, :]
                        nc.tensor.transpose(pt, src_t, ident_sb)
                        dst = qd_j[:, h, jj:jj + 128] if h < NH \
                            else kd_j[:, jj:jj + 128]
                        if h % 2 == 0:
                            nc.scalar.activation(dst, pt, AF.Copy)
                        else:
                            nc.vector.tensor_copy(dst, pt)

                for i in range(NT):
                    if i + 3 < NT:
                        load_x(i + 3)
                        if i + 4 == NT:
                            # SP queue is free after the x stream: pull in
                            # Wo for P3 while P1/P2 still run
                            for og in range(4):
                                nc.sync.dma_start(
                                    out=wo_sb[:, :, og * 512:(og + 1) * 512],
                                    in_=wod[:, :, og * 512:(og + 1) * 512],
                                )
                    xk = xks.pop(i)
                    ps_q = p1ps.tile([128, 512], F32, tag="psq")
                    ps_kv = p1ps.tile([128, 256], F32, tag="pskv")
                    for k in range(KT):
                        nc.tensor.matmul(
                            ps_q, xk[:, k * 128:(k + 1) * 128], wqkv[:, k, 0:512],
                            start=(k == 0), stop=(k == KT - 1),
                        )
                    for k in range(KT):
                        nc.tensor.matmul(
                            ps_kv, xk[:, k * 128:(k + 1) * 128], wqkv[:, k, 512:768],
                            start=(k == 0), stop=(k == KT - 1),
                        )
                    pending[i] = (ps_q, ps_kv)
                    if i > 0:
                        p1_tail(i - 1)
                    if i == 14:
                        # head-0's first blocks only need q/k tiles 0-7:
                        # emit them now so their exp/mask ops sit ahead of
                        # the last P1 tail chains in the ACT/DVE queues
                        for kj0 in range(4):
                            ets0[kj0] = emit_scores(0, kj0)
                p1_tail(NT - 1)

            # ---------------- Phase 2: windowed attention --------------------
            with tc.tile_pool(name="p2po", bufs=2, space="PSUM") as p2po, \
                 tc.tile_pool(name="p2pd", bufs=2, space="PSUM") as p2pd:

                for h in range(NH):
                    ets = dict(ets0) if h == 0 else {}
                    for kj in range(NT):
                        if kj not in ets:
                            ets[kj] = emit_scores(h, kj)

                        if kj % 4 != 3:
                            continue
                        g = kj // 4
                        ps_o = p2po.tile([128, 512], F32, tag="o")
                        ps_d = p2pd.tile([128, 512], F32, tag="d")
                        pieces = []
                        for kj2 in range(max(0, 4 * g - omax),
                                         min(NT - 1, 4 * g + 3) + 1):
                            lo = max(4 * g, kj2)
                            hi = min(4 * g + 3, kj2 + omax, NT - 1)
                            if lo > hi:
                                continue
                            pieces.append((kj2, lo, hi))
                        # hardware tracks has_written per element: the first
                        # matmul clears the bank, later ones overwrite
                        # untouched cols and accumulate the rest, so
                        # overlapping pieces are legal and stay wide.
                        last_i = len(pieces) - 1
                        for idx, (kj2, lo, hi) in enumerate(pieces):
                            n = (hi - lo + 1) * 128
                            to = (lo - kj2) * 128
                            po = (lo - 4 * g) * 128
                            vs = vsb_a[:, kj2, :] if kj2 < 8 \
                                else vsb_b[:, kj2 - 8, :]
                            nc.tensor.matmul(
                                ps_o[:, po:po + n], vs,
                                ets[kj2][:, to:to + n],
                                start=(idx == 0), stop=(idx == last_i),
                                skip_group_check=True,
                            )
                        for idx, (kj2, lo, hi) in enumerate(pieces):
                            n = (hi - lo + 1) * 128
                            to = (lo - kj2) * 128
                            po = (lo - 4 * g) * 128
                            nc.tensor.matmul(
                                ps_d[:, po:po + n], ones_sb,
                                ets[kj2][:, to:to + n],
                                start=(idx == 0), stop=(idx == last_i),
                                skip_group_check=True,
                            )
                        rec = p2s.tile([128, 512], F32, tag="rec")
                        nc.vector.reciprocal(rec, ps_d)
                        nc.vector.tensor_mul(yTg[g][:, h, :], ps_o, rec)


            # ---------------- Phase 3: c_proj partial ------------------------
            with tc.tile_pool(name="p3o", bufs=4) as p3o, \
                 tc.tile_pool(name="p3ps", bufs=4, space="PSUM") as p3ps:
                for i in range(NT):
                    for og in range(C // 512):
                        ps = p3ps.tile([128, 512], F32, tag="po")
                        for h in range(NH):
                            nc.tensor.matmul(
                                ps,
                                yTg[i // 4][:, h, (i % 4) * 128:
                                            (i % 4 + 1) * 128],
                                wo_sb[:, h, og * 512:(og + 1) * 512],
                                start=(h == 0), stop=(h == NH - 1),
                            )
                        # alternate copy engines so neither DVE nor ACT
                        # gates the PE accumulation pipeline
                        ot = p3o.tile([128, 512], F32, tag="ot")
                        if og % 2 == 0:
                            nc.vector.tensor_copy(ot, ps)
                        else:
                            nc.scalar.activation(ot, ps, AF.Copy)
                        nc.sync.dma_start(
                            out=outp[i * 128:(i + 1) * 128,
                                     og * 512:(og + 1) * 512],
                            in_=ot)

            _p2ps_cm.__exit__(None, None, None)
            _p2s_cm.__exit__(None, None, None)
            _p2e_cm.__exit__(None, None, None)

    nc.compile()
    return nc, mask_off, nm


_CACHE = {}


def _get_nc(W: int):
    if W not in _CACHE:
        _CACHE[W] = build_nc(W)
    return _CACHE[W]


def kernel(x, cos, sin, Wq, Wk, Wv, Wo, window_left):
    bf = mybir.dt.np(BF)
    x = np.asarray(x, dtype=np.float32)
    cos = np.asarray(cos, dtype=np.float32).reshape(T, 64)
    sin = np.asarray(sin, dtype=np.float32).reshape(T, 64)
    Wq = np.asarray(Wq, dtype=np.float32)
    Wk = np.asarray(Wk, dtype=np.float32)
    Wv = np.asarray(Wv, dtype=np.float32)
    Wo = np.asarray(Wo, dtype=np.float32)
    W = int(np.asarray(window_left))

    nc, mask_off, nm = _get_nc(W)
    _, mask_arrs = _window_masks(W)
    masks_np = np.zeros((nm, 128, 128), dtype=np.float32)
    for i, o in enumerate(mask_off):
        masks_np[i] = mask_arrs[o]
    maskp = np.ascontiguousarray(
        masks_np.transpose(1, 0, 2)).astype(bf)  # [p, m, q]

    ones_np = np.ones((128, 128), dtype=bf)
    ident_np = np.eye(128, dtype=np.float32).astype(bf)
    cosp = np.ascontiguousarray(
        cos.reshape(NT, 128, 64).transpose(1, 0, 2)).astype(bf)
    sinp = np.ascontiguousarray(
        sin.reshape(NT, 128, 64).transpose(1, 0, 2)).astype(bf)

    # [i, p, k, t] = x[b, i*128+t, k*128+p]
    xps = [
        np.ascontiguousarray(
            x[b].reshape(NT, 128, KT, 128).transpose(0, 3, 2, 1)
            .reshape(NT, 128, KT * 128)).astype(bf)
        for b in range(B)
    ]
    in_maps = []
    for c in range(8):
        b, g = c // 4, c % 4
        wcat = np.concatenate(
            [Wq[512 * g:512 * (g + 1)], Wk[128 * g:128 * (g + 1)],
             Wv[128 * g:128 * (g + 1)]], axis=0
        )  # [768, 2048]
        wp = np.ascontiguousarray(
            wcat.T.reshape(KT, 128, FEAT)).astype(bf)
        wop = np.ascontiguousarray(
            Wo[:, 512 * g:512 * (g + 1)].T
            .reshape(NH, 128, C).transpose(1, 0, 2)).astype(bf)
        in_maps.append({
            "xd": xps[b],
            "wd": wp,
            "wod": wop,
            "cosd": cosp,
            "sind": sinp,
            "maskd": maskp,
            "onesd": ones_np,
            "identd": ident_np,
        })

    trace = os.environ.get("KERNEL_TRACE") == "1"
    try:
        res = run_bass_kernel_spmd(nc, in_maps, core_ids=list(range(8)),
                                   trace=trace)
    except ModuleNotFoundError:
        # NTFF profile hook unavailable in this container - run untraced
        res = run_bass_kernel_spmd(nc, in_maps, core_ids=list(range(8)))
    global LAST_EXEC_NS
    LAST_EXEC_NS = res.exec_time_ns
    out = np.zeros((B, T, C), dtype=np.float32)
    for c in range(8):
        out[c // 4] += res.results[c]["outp"]
    return out


LAST_EXEC_NS = None
